# revision 1
# baseline (speedup 1.0000x reference)
"""Multi-head attention (B=8, N=1024, C=768, H=12) on 8 TRN2 NeuronCores.

Sharding: pure data parallel — batch element b runs on core b. No collectives.

Per-core pipeline (all matmuls bf16 on TensorE, fp32 PSUM accumulation):
  1. x [1024,768] f32 -> TensorE-transpose -> xT bf16 [768,1024]
  2. q,k projections computed TRANSPOSED: qkT[f, s] via lhsT=Wqk (native
     layout), rhs=xT — so attention scores need no further transposes.
     Only pair-0 tiles are computed up front; the remaining 10 feature
     tiles are interleaved into the attention loop as PE filler.
  3. v[s, f] natural via lhsT=xT, rhs=Wv, stored per head as [v | 1]:
     the ones column folds the softmax denominator into the PV matmul
     as an extra output row (row 64 of the PSUM accumulator).
  4. attention as ONE flat software pipeline over all 96 stages
     (pair t, q-half, k-tile): scores^T [k,q] = kT.T @ qT with K=64
     row-paired heads (tile_position (0,0)/(64,0) — verified ~1.6x
     concurrency on silicon); one ScalarE exp per stage covers both
     heads ([128,1024], no max subtraction: scores ~N(0, 0.31), safe);
     PV accumulates outT[d,q] + sum row over k-tiles in PSUM; the next
     stage's scores are issued before PV(s) so the exp stream on the
     ScalarE never starves.
  5. normalize, deferred off the critical path: fast PSUM->SBUF copies
     release the accumulators; reciprocal of the sum row; partition
     broadcast via DRAM-bounce DMA (last pair: K=1 ones-matmul on the
     idle TensorE instead); multiplies emitted two stages later so the
     DVE queue never blocks on the DMA round trip. Head B is partition
     shifted 0:64 -> 64:128 by a SBUF->SBUF DMA so attn_outT has the
     proj lhsT layout directly.
  6. proj: out[s, f] = attn_outT.T @ Wproj + b, stores fanned over 3
     DMA queues.

PSUM budget (8 banks): 2 fill (interleaved qk projections) + 4 scores
(double-buffered [128,1024]) + 2 PV accumulators.
"""
import sys

if "/opt/trn_rl_repo" not in sys.path:
    sys.path.insert(0, "/opt/trn_rl_repo")

from contextlib import ExitStack

import numpy as np

import concourse.bass as bass
import concourse.tile as tile
from concourse import mybir
from concourse.bass_utils import run_bass_kernel_spmd
from concourse.masks import make_identity

FP32 = mybir.dt.float32
BF16 = mybir.dt.bfloat16
Exp = mybir.ActivationFunctionType.Exp

S = 1024          # sequence length (per core batch element)
C = 768           # model dim
H = 12            # heads
HD = 64           # head dim
C3 = 3 * C
P = 128
ST = S // P       # 8 seq tiles
CT = C // P       # 6 feature tiles
MT = 12           # q+k feature tiles of qkT
PAIRS = H // 2    # 6 head pairs
SCALE = HD ** -0.5
N_CORES = 8


def split_multiwait(nc, max_waits=1):
    """This walrus build rejects instructions with >1 semaphore waits (the
    Tile kernel-tail Drain accumulates one per live proc). Split extras into
    chained Drains on the same engine immediately before."""
    for func in nc.m.functions:
        for block in func.blocks:
            newlist = []
            for ins in block.instructions:
                si = ins.sync_info
                if si is not None and si.on_wait is not None and len(si.on_wait) > max_waits:
                    waits = list(si.on_wait)
                    extra, keep = waits[:-max_waits], waits[-max_waits:]
                    for j, w in enumerate(extra):
                        nd = mybir.InstDrain(
                            name=f"{ins.name}-wsplit{j}",
                            engine=ins.engine,
                            ins=[], outs=[],
                            sync_info=mybir.SyncInfo(on_wait=[w], on_update=[]),
                        )
                        newlist.append(nd)
                        nc.inst_map[nd.name] = nd
                    ins.sync_info = mybir.SyncInfo(
                        on_wait=keep, on_update=list(si.on_update or [])
                    )
                newlist.append(ins)
            block.instructions = newlist


def build_nc(reps=1):
    nc = bass.Bass()
    x_ext = nc.declare_dram_parameter("x", [S, C], FP32, isOutput=False)
    qkvw_ext = nc.declare_dram_parameter("qkv_w", [C, C3], FP32, isOutput=False)
    qkvb_ext = nc.declare_dram_parameter("qkv_b", [C3], FP32, isOutput=False)
    projw_ext = nc.declare_dram_parameter("proj_w", [C, C], FP32, isOutput=False)
    projb_ext = nc.declare_dram_parameter("proj_b", [C], FP32, isOutput=False)
    out_ext = nc.declare_dram_parameter("out", [S, C], FP32, isOutput=True)

    with tile.TileContext(nc) as tc, ExitStack() as ctx:
        consts = ctx.enter_context(tc.tile_pool(name="consts", bufs=1))
        wpool = ctx.enter_context(tc.tile_pool(name="weights", bufs=1))
        xpool = ctx.enter_context(tc.tile_pool(name="xpool", bufs=1))
        actpool = ctx.enter_context(tc.tile_pool(name="actpool", bufs=1))
        ptpool = ctx.enter_context(tc.tile_pool(name="ptpool", bufs=2))
        rpool = ctx.enter_context(tc.tile_pool(name="rpool", bufs=1))
        opool = ctx.enter_context(tc.tile_pool(name="opool", bufs=2))

        # ---- x load first: the first transposes are the first PE work, so
        # the x chunks go at the head of the scalar HWDGE queue (low issue
        # latency); everything below on this queue is needed later.
        stage = ctx.enter_context(tc.tile_pool(name="stage", bufs=2))
        xfall = stage.tile([P, ST, C], FP32, tag="xfall", bufs=1)
        half_st = ST // 2

        def load_x(eng):
            for hc in range(2):
                eng.dma_start(
                    out=xfall[:, hc * half_st:(hc + 1) * half_st, :],
                    in_=bass.AP(tensor=x_ext, offset=hc * half_st * P * C,
                                ap=[[C, P], [P * C, half_st], [1, C]]))

        # ---- constants / biases ----
        ident = consts.tile([P, P], FP32, tag="ident")
        make_identity(nc, ident)
        ones_r = consts.tile([P, HD], BF16, tag="ones_r")
        nc.vector.memset(ones_r[HD:HD + 1, :], 1.0)

        # qk bias: feature tile mt -> column mt, features on partitions
        qkb = consts.tile([P, MT], FP32, tag="qkb")
        qkb_src = bass.AP(tensor=qkvb_ext, offset=0, ap=[[1, P], [P, MT]])
        nc.scalar.dma_start(out=qkb, in_=qkb_src)
        # v bias broadcast to all partitions [128, 768]
        vb = consts.tile([P, C], FP32, tag="vb")
        vb_src = bass.AP(tensor=qkvb_ext, offset=2 * C, ap=[[0, P], [1, C]])
        nc.scalar.dma_start(out=vb, in_=vb_src)
        # proj bias broadcast
        pb = consts.tile([P, C], FP32, tag="pb")
        pb_src = bass.AP(tensor=projb_ext, offset=0, ap=[[0, P], [1, C]])
        nc.scalar.dma_start(out=pb, in_=pb_src)

        # ---- load + cast weights (once) ----
        # weights on the sync HWDGE queue, x on the gpsimd queue so the
        # 9.3 MB of weights don't serialize in front of x (transposes are
        # the first PE work).
        xT = [xpool.tile([P, S], BF16, tag=f"xT{ct}", name=f"xT{ct}") for ct in range(CT)]
        wproj = []
        if True:
            # q,k weight columns on the sync queue, v columns + proj on the
            # scalar queue: the attention prefix is gated on q,k (then v)
            # weights, so halve the critical DMA stream.
            wqkv = [wpool.tile([P, C3], BF16, tag=f"wqkv{ct}", name=f"wqkv{ct}")
                    for ct in range(CT)]
            for chunk in range(3):
                cts = range(chunk * 2, chunk * 2 + 2)
                stg = stage.tile([P, 2, 2 * C], FP32, tag="wstg")
                # one DMA per 2 feature tiles: rows ct*128+p, cols 0:1536
                nc.sync.dma_start(out=stg, in_=bass.AP(
                    tensor=qkvw_ext, offset=chunk * 2 * P * C3,
                    ap=[[C3, P], [P * C3, 2], [1, 2 * C]]))
                for j, ct in enumerate(cts):
                    nc.scalar.copy(out=wqkv[ct][:, 0:2 * C], in_=stg[:, j, :])
            for chunk in range(3):
                cts = range(chunk * 2, chunk * 2 + 2)
                stgv = stage.tile([P, 2, C], FP32, tag="wstgv", bufs=1)
                nc.scalar.dma_start(out=stgv, in_=bass.AP(
                    tensor=qkvw_ext, offset=chunk * 2 * P * C3 + 2 * C,
                    ap=[[C3, P], [P * C3, 2], [1, C]]))
                for j, ct in enumerate(cts):
                    nc.vector.tensor_copy(out=wqkv[ct][:, 2 * C:], in_=stgv[:, j, :])
            def load_proj_weights():
                # emitted late (after attention) so these DMAs/casts run in
                # attention-window slack, not in the prefix critical path
                for ct in range(CT):
                    stg = stage.tile([P, C], FP32, tag="pstg", name=f"pstg{ct}", bufs=1)
                    nc.gpsimd.dma_start(out=stg, in_=projw_ext[ct * P:(ct + 1) * P, :])
                    w = wpool.tile([P, C], BF16, tag=f"wproj{ct}", name=f"wproj{ct}")
                    nc.vector.tensor_copy(out=w, in_=stg)
                    wproj.append(w)

      # body (repeatable for timing builds; reps=1 for grading)
        _body_indent_marker = None
        dscr = ctx.enter_context(tc.tile_pool(name="dscr", bufs=2, space="DRAM"))
        ps_fill = ctx.enter_context(tc.tile_pool(name="ps_fill", bufs=2, space="PSUM"))

        # ---- per-rep body: load x, transpose, qkv, attention, proj ----
        for _rep in range(reps):
          with tc.tile_pool(name="ps_tp", bufs=2, space="PSUM") as ps_tp:
            load_x(nc.gpsimd)
            xf = [xfall[:, st, :] for st in range(ST)]
            for sg in range(2):
                for ct in range(CT):
                    pt = ps_tp.tile([P, 4 * P], FP32, tag="tps")
                    for j in range(4):
                        st = sg * 4 + j
                        nc.tensor.transpose(
                            out=pt[:, j * P:(j + 1) * P],
                            in_=xf[st][:, ct * P:(ct + 1) * P],
                            identity=ident,
                        )
                    nc.vector.tensor_copy(
                        out=xT[ct][:, sg * 512:(sg + 1) * 512], in_=pt
                    )
          # ---- q,k projections (transposed), v (natural), attention ----
          # qk feature tiles: q head h lives in qk[h//2] rows (h%2)*64..+64,
          # k head h in qk[PAIRS + h//2] same rows. Only pair 0's tiles are
          # computed up front; the rest are interleaved into the attention
          # loop as PE filler during ACT(exp)-bound stretches (ps_fill keeps
          # 2 PSUM banks reserved for this through the attention phase).
          qk = [actpool.tile([P, S], BF16, tag=f"qk{mt}", name=f"qk{mt}") for mt in range(MT)]
          # v tiles: [128, head, 65] = [v_h + bias | ones]; the ones column
          # folds the softmax denominator into the PV matmul as out row 64.
          vsb = [actpool.tile([P, H, HD + 1], BF16, tag=f"v{st}", name=f"v{st}") for st in range(ST)]
          aoT = [actpool.tile([P, S], BF16, tag=f"aoT{t}", name=f"aoT{t}") for t in range(PAIRS)]


          def qk_tile_steps(mt):
              """Generator: 12 matmuls for feature tile mt, yielding after each
              so they can be interleaved with attention as PE filler."""
              for nh in range(2):
                  ps = ps_fill.tile([P, 512], FP32, tag="qkps")
                  for ct in range(CT):
                      nc.tensor.matmul(
                          ps,
                          lhsT=wqkv[ct][:, mt * P:(mt + 1) * P],
                          rhs=xT[ct][:, nh * 512:(nh + 1) * 512],
                          start=(ct == 0), stop=(ct == CT - 1),
                      )
                      if ct == CT - 1:
                          nc.vector.tensor_scalar_add(
                              out=qk[mt][:, nh * 512:(nh + 1) * 512],
                              in0=ps, scalar1=qkb[:, mt:mt + 1],
                          )
                      yield

          # prefix: v first (its casts arrive first), then pair-0 qk tiles
          with tc.tile_pool(name="ps_v", bufs=2, space="PSUM") as ps_v:
              for st in range(ST):
                  ps = ps_v.tile([P, C], FP32, tag="vps")
                  for half in range(2):
                      sl = slice(half * 512, min((half + 1) * 512, C))
                      for ct in range(CT):
                          nc.tensor.matmul(
                              ps[:, sl],
                              lhsT=xT[ct][:, st * P:(st + 1) * P],
                              rhs=wqkv[ct][:, 2 * C + sl.start: 2 * C + sl.stop],
                              start=(ct == 0), stop=(ct == CT - 1),
                          )
                  nc.vector.tensor_add(
                      out=vsb[st][:, :, 0:HD],
                      in0=ps.rearrange("p (h d) -> p h d", d=HD),
                      in1=vb.rearrange("p (h d) -> p h d", d=HD),
                  )
                  nc.vector.memset(vsb[st][:, :, HD:HD + 1], 1.0)
          for mt in (0, PAIRS):
              for _ in qk_tile_steps(mt):
                  pass

          # remaining qk tiles, interleaved: during pair t compute pair t+1
          def pair_fills(t2):
              if t2 >= PAIRS:
                  return iter(())
              def gen():
                  yield from qk_tile_steps(t2)
                  yield from qk_tile_steps(PAIRS + t2)
              return gen()

          pending_muls = []
          with tc.tile_pool(name="ps_attn", bufs=1, space="PSUM") as ps_attn:
              # Flat software pipeline over all (pair, qh, kk) stages:
              # scores(s+1) is issued before PV(s) so the PE never waits on
              # exp at qh/pair boundaries.
              stages = [(t, qh, kk)
                        for t in range(PAIRS) for qh in range(2) for kk in range(ST)]
              sab_tiles = {}
              fills = {t: pair_fills(t + 1) for t in range(PAIRS)}

              def emit_scores(s):
                  t, qh, kk = s
                  qsl = slice(qh * 512, (qh + 1) * 512)
                  ksl = slice(kk * P, (kk + 1) * P)
                  sAB = ps_attn.tile([P, S], FP32, tag="sAB", bufs=2, name=f"sAB{t}_{qh}_{kk}")
                  nc.tensor.matmul(
                      sAB[:, 0:512],
                      lhsT=qk[PAIRS + t][0:HD, ksl], rhs=qk[t][0:HD, qsl],
                      start=True, stop=True, tile_position=(0, 0),
                  )
                  nc.tensor.matmul(
                      sAB[:, 512:1024],
                      lhsT=qk[PAIRS + t][HD:P, ksl], rhs=qk[t][HD:P, qsl],
                      start=True, stop=True, tile_position=(HD, 0),
                  )
                  sab_tiles[s] = sAB

              emit_scores(stages[0])
              oA = oB = None
              for i, s in enumerate(stages):
                  t, qh, kk = s
                  sAB = sab_tiles.pop(s)
                  pt = ptpool.tile([P, S], BF16, tag="pt", bufs=3)
                  nc.scalar.activation(out=pt, in_=sAB, func=Exp, scale=SCALE)
                  if i + 1 < len(stages):
                      emit_scores(stages[i + 1])
                  if kk == 0:
                      oA = ps_attn.tile([HD + 1, 512], FP32, tag="oA", name=f"oA{t}_{qh}")
                      oB = ps_attn.tile([HD + 1, 512], FP32, tag="oB", name=f"oB{t}_{qh}")
                  nc.tensor.matmul(
                      oA, lhsT=vsb[kk][:, 2 * t, :], rhs=pt[:, 0:512],
                      start=(kk == 0), stop=(kk == ST - 1),
                  )
                  nc.tensor.matmul(
                      oB, lhsT=vsb[kk][:, 2 * t + 1, :], rhs=pt[:, 512:1024],
                      start=(kk == 0), stop=(kk == ST - 1),
                  )
                  # PE filler: next pair's qk projection matmuls, spread
                  # ~1.6/stage so no stage is badly PE-over-paced; all 24
                  # complete by stage 14 (the next pair's first scores are
                  # emitted during stage 15).
                  stage_in_pair = (qh * ST + kk)
                  nfill = 2 if stage_in_pair % 3 != 2 else 1
                  for _ in range(nfill):
                      next(fills[t], None)
                  if kk != ST - 1:
                      continue
                  # stage completes a (t, qh) slice: copy PSUM out fast
                  # (incl. sum row 64) so the next PV isn't gated on the
                  # normalize chain, then deferred-normalize.
                  qsl = slice(qh * 512, (qh + 1) * 512)
                  uA = rpool.tile([P, 512], FP32, tag="uA", bufs=3)
                  uB = rpool.tile([P, 512], FP32, tag="uB", bufs=3)
                  nc.vector.tensor_copy(out=uA[0:HD + 1, :], in_=oA)
                  nc.vector.tensor_copy(out=uB[0:HD + 1, :], in_=oB)
                  uBs = rpool.tile([P, 512], FP32, tag="uBs", bufs=3)
                  if t == PAIRS - 1:
                      # tail fast-path: reciprocal straight to bf16 and
                      # broadcast across partitions with a K=1 TensorE matmul
                      # into the (idle) fill-PSUM slots instead of the
                      # DRAM-bounce DMA chain; shift DMA on the low-latency
                      # HWDGE queue. bf16 denominators only on this pair
                      # (~0.4% on 1/6 of heads).
                      nc.vector.reciprocal(out=uA[HD:HD + 1, :], in_=uA[HD:HD + 1, :])
                      nc.vector.reciprocal(out=uB[HD:HD + 1, :], in_=uB[HD:HD + 1, :])
                      nc.sync.dma_start(out=uBs[HD:P, :], in_=uB[0:HD, :])
                      rbf = rpool.tile([P, 512], BF16, tag="rbf", bufs=1)
                      nc.vector.tensor_copy(out=rbf[HD:HD + 1, :], in_=uA[HD:HD + 1, :])
                      rbfB = rpool.tile([P, 512], BF16, tag="rbfB", bufs=1)
                      nc.vector.tensor_copy(out=rbfB[HD:HD + 1, :], in_=uB[HD:HD + 1, :])
                      rAp = ps_fill.tile([HD, 512], FP32, tag="qkps", name=f"rAp{qh}")
                      nc.tensor.matmul(
                          rAp, lhsT=ones_r[HD:HD + 1, 0:HD], rhs=rbf[HD:HD + 1, :],
                          start=True, stop=True, tile_position=(HD, 0),
                      )
                      rBp = ps_fill.tile([P, 512], FP32, tag="qkps", name=f"rBp{qh}")
                      nc.tensor.matmul(
                          rBp[HD:P, :], lhsT=ones_r[HD:HD + 1, 0:HD], rhs=rbfB[HD:HD + 1, :],
                          start=True, stop=True, tile_position=(HD, HD),
                      )
                      nc.vector.tensor_mul(
                          out=aoT[t][0:HD, qsl], in0=uA[0:HD, :], in1=rAp[0:HD, :]
                      )
                      nc.vector.tensor_mul(
                          out=aoT[t][HD:P, qsl], in0=uBs[HD:P, :], in1=rBp[HD:P, :]
                      )
                  else:
                      nc.vector.reciprocal(out=uA[HD:HD + 1, :], in_=uA[HD:HD + 1, :])
                      nc.vector.reciprocal(out=uB[HD:HD + 1, :], in_=uB[HD:HD + 1, :])
                      nc.gpsimd.dma_start(out=uBs[HD:P, :], in_=uB[0:HD, :])
                      rA = rpool.tile([P, 512], FP32, tag="rA", bufs=3)
                      rB = rpool.tile([P, 512], FP32, tag="rB", bufs=3)
                      dA = dscr.tile([512], FP32, tag="dA")
                      dB = dscr.tile([512], FP32, tag="dB")
                      nc.sync.dma_start(out=dA, in_=uA[HD:HD + 1, :])
                      nc.gpsimd.dma_start(out=dB, in_=uB[HD:HD + 1, :])
                      nc.sync.dma_start(
                          out=rA[0:HD, :],
                          in_=bass.AP(tensor=dA.tensor, offset=dA.offset, ap=[[0, HD], [1, 512]]),
                      )
                      nc.gpsimd.dma_start(
                          out=rB[HD:P, :],
                          in_=bass.AP(tensor=dB.tensor, offset=dB.offset, ap=[[0, HD], [1, 512]]),
                      )
                      def emit_muls(t=t, qsl=qsl, uA=uA, rA=rA, uBs=uBs, rB=rB):
                          nc.vector.tensor_mul(
                              out=aoT[t][0:HD, qsl], in0=uA[0:HD, :], in1=rA[0:HD, :]
                          )
                          nc.vector.tensor_mul(
                              out=aoT[t][HD:P, qsl], in0=uBs[HD:P, :], in1=rB[HD:P, :]
                          )
                      pending_muls.append(emit_muls)
                      if len(pending_muls) > 2:
                          pending_muls.pop(0)()
                  if qh == 1:
                      for _ in fills[t]:
                          pass

          for m in pending_muls:
              m()
          pending_muls.clear()
          if not wproj:
              load_proj_weights()

          # ---- proj ----
          with tc.tile_pool(name="ps_proj", bufs=2, space="PSUM") as ps_proj:
              for st in range(ST):
                  ps = ps_proj.tile([P, C], FP32, tag="prps", bufs=3)
                  for half in range(2):
                      sl = slice(half * 512, min((half + 1) * 512, C))
                      for ct in range(CT):
                          nc.tensor.matmul(
                              ps[:, sl],
                              lhsT=aoT[ct][:, st * P:(st + 1) * P],
                              rhs=wproj[ct][:, sl],
                              start=(ct == 0), stop=(ct == CT - 1),
                          )
                  ost = opool.tile([P, C], FP32, tag="ostg", bufs=3)
                  nc.vector.tensor_add(out=ost, in0=ps, in1=pb)
                  eng = (nc.scalar, nc.sync, nc.gpsimd)[st % 3]
                  eng.dma_start(out=out_ext[st * P:(st + 1) * P, :], in_=ost)


    split_multiwait(nc)
    return nc


_NC_CACHE = None


def get_nc():
    global _NC_CACHE
    if _NC_CACHE is None:
        _NC_CACHE = build_nc()
    return _NC_CACHE


def kernel(x, qkv_w, qkv_b, proj_w, proj_b):
    x = np.ascontiguousarray(np.asarray(x, dtype=np.float32))
    in_common = {
        "qkv_w": np.ascontiguousarray(np.asarray(qkv_w, dtype=np.float32)),
        "qkv_b": np.ascontiguousarray(np.asarray(qkv_b, dtype=np.float32)),
        "proj_w": np.ascontiguousarray(np.asarray(proj_w, dtype=np.float32)),
        "proj_b": np.ascontiguousarray(np.asarray(proj_b, dtype=np.float32)),
    }
    in_maps = [{"x": x[b], **in_common} for b in range(N_CORES)]
    nc = get_nc()
    res = run_bass_kernel_spmd(nc, in_maps, core_ids=list(range(N_CORES)))
    return np.stack([res.results[b]["out"] for b in range(N_CORES)], axis=0)

